# revision 7
# baseline (speedup 1.0000x reference)
"""Trainium2 Bass kernel for nn_Corr (correlation-attention module).

Math (per batch n):
    f1 = 0.5*(w1 @ feat + b1)        # [4, 6400]   feat = feature_in[n] flattened
    f2 =      w2 @ feat + b2         # [4, 6400]
    S  = f1^T @ f2                   # [6400, 6400]  (0.5 = 1/sqrt(nclass) folded into f1)
    A  = softmax(S, axis=1)          # row softmax (over q)
    V  = bilinear_resize(out[n])     # [4, 6400]
    fina[c, q] = sum_p V[c, p]/Z_p * exp(S[p, q])

Sharding: 2 batches x 4 p-shards (rows of S) = 8 cores. Each core produces a
partial fina over its 1600 p-rows; host sums the 4 partials per batch.

Device kernel per core (p-shard of 1664 rows incl. pad, all 6400 q):
  - S chunk = matmul(lhsT=f1pad[:, pblock(128 cols)], rhs=f2pad[:, qchunk])
    single M=128 PE tile, N=512 chunks into PSUM round buffers (2x1024 f32).
  - exp on ScalarE PSUM->SBUF (fp16 E strip) with accum_out giving row sums Z.
  - fina = matmul(lhsT=vt*recip(Z), rhs=E) col-tiled 4x (q-group g on PE col
    group g / PSUM partition quadrant g), accumulated IN PSUM across all 13
    p-blocks (start on pb==0, stop on pb==12); one DVE copy + DMA at the end.
  - emit order pipelines fina(pb-1) between S(pb) rounds so the PE never
    waits on the exp of the current block. ScalarE (exp) is the bottleneck
    engine; everything else hides under it.
"""

import numpy as np

N_CORES = 8
NB = 2          # batches
NCLS = 4        # nclass
C_IN = 32
H = W = 80
HW = H * W      # 6400
NSH = 4         # p-shards per batch
PSH = HW // NSH  # 1600 p rows per shard
PBLK = 13        # p blocks of 128 (1664 = 13*128, last 64 rows are zero-pad)
PPAD = PBLK * 128  # 1664
QCH = 512        # q chunk (psum bank)
ROUND = 1024     # q per exp round (2 psum banks)
NROUND = 7       # 6*1024 + 256

_CACHE = {}


def _resize_bilinear_ac(x, h_out, w_out):
    """numpy mirror of the reference's align_corners=True bilinear resize."""
    n, c, h, w = x.shape
    if (h, w) == (h_out, w_out):
        return x
    ys = np.linspace(0.0, h - 1.0, h_out, dtype=np.float32)
    xs = np.linspace(0.0, w - 1.0, w_out, dtype=np.float32)
    y0 = np.floor(ys).astype(np.int32)
    x0 = np.floor(xs).astype(np.int32)
    y1 = np.minimum(y0 + 1, h - 1)
    x1 = np.minimum(x0 + 1, w - 1)
    wy = (ys - y0.astype(np.float32))[None, None, :, None]
    wx = (xs - x0.astype(np.float32))[None, None, None, :]
    g = lambda yi, xi: x[:, :, yi, :][:, :, :, xi]
    top = g(y0, x0) * (1.0 - wx) + g(y0, x1) * wx
    bot = g(y1, x0) * (1.0 - wx) + g(y1, x1) * wx
    return (top * (1.0 - wy) + bot * wy).astype(np.float32)


def _build_bass():
    import concourse.bass as bass
    import concourse.tile as tile
    from concourse import bacc, mybir

    f32 = mybir.dt.float32
    f16 = mybir.dt.float16

    nc = bacc.Bacc(
        "TRN2", target_bir_lowering=False, debug=False, num_devices=N_CORES
    )

    f1p_d = nc.dram_tensor("f1p", [12, PPAD], f16, kind="ExternalInput")
    f2p_d = nc.dram_tensor("f2p", [12, HW], f16, kind="ExternalInput")
    vt_d = nc.dram_tensor("vt", [128, NCLS * PBLK], f32, kind="ExternalInput")
    res_d = nc.dram_tensor("res", [4 * NCLS, 4 * QCH], f32, kind="ExternalOutput")

    EXP = mybir.ActivationFunctionType.Exp
    ADD = mybir.AluOpType.add
    MULT = mybir.AluOpType.mult
    AXX = mybir.AxisListType.X

    with tile.TileContext(nc) as tc:
        with (
            tc.tile_pool(name="const", bufs=1) as cpool,
            tc.tile_pool(name="estrip", bufs=2) as epool,
            tc.tile_pool(name="zpool", bufs=2) as zpool,
            tc.tile_pool(name="spsum", bufs=2, space="PSUM") as spool,
            tc.tile_pool(name="fpsum", bufs=1, space="PSUM") as fpool,
        ):
            # K=12 contraction: the PE only contracts the AP's partition
            # count, so no zero-padding of the contraction dim is needed
            f1s = cpool.tile([12, PPAD], f16, tag="f1s")
            f2s = cpool.tile([12, HW], f16, tag="f2s")
            vts = cpool.tile([128, NCLS * PBLK], f32, tag="vts")
            accsb = cpool.tile([128, 4 * QCH], f32, tag="accsb")
            # ping-pong [128, 32] lhsT tiles for fina (cols 4..31 stay zero)
            vtpA = cpool.tile([128, 32], f16, tag="vtpA")
            vtpB = cpool.tile([128, 32], f16, tag="vtpB")
            bneg = cpool.tile([128, 1], f32, tag="bneg")
            scr = cpool.tile([128, 1], f32, tag="scr")
            # persistent fina accumulator: 4 PSUM banks, accumulated over all
            # p-blocks by the PE itself (start only on the first block)
            fp = fpool.tile([128, 4 * QCH], f32, tag="fp")

            nc.gpsimd.memset(vtpA[:, :], 0.0)
            nc.gpsimd.memset(vtpB[:, :], 0.0)
            nc.gpsimd.memset(bneg[:, :], -5.0)
            # dummy exp so the ~2.7us ACT table load overlaps the input DMAs
            nc.scalar.activation(scr[:, 0:1], bneg[:, 0:1], EXP)
            nc.sync.dma_start(out=f1s[:, :], in_=f1p_d[:, :])
            nc.sync.dma_start(out=f2s[:, :], in_=f2p_d[:, :])
            nc.sync.dma_start(out=vts[:, :], in_=vt_d[:, :])

            ets = [None] * PBLK
            vtps = [None] * PBLK

            def emit_fina_wave(pb, t):
                # col-tiled 4x: group g covers q in [1600g, 1600g+1600) as
                # waves of N=512,512,512,64; PSUM bank t cols 512t..; group g
                # on partition quadrant 32g. Accumulate across p-blocks.
                et = ets[pb]
                vtp = vtps[pb]
                qw = 512 if t < 3 else 64
                for g in range(4):
                    qo = 1600 * g + QCH * t
                    nc.tensor.matmul(
                        fp[32 * g : 32 * g + 32, QCH * t : QCH * t + qw],
                        lhsT=vtp[:, :],
                        rhs=et[:, qo : qo + qw],
                        start=(pb == 0),
                        stop=(pb == PBLK - 1),
                        tile_position=(0, 32 * g),
                        skip_group_check=True,
                    )

            for pb in range(PBLK):
                et = epool.tile([128, HW], f16, tag="et")
                zparts = zpool.tile([128, NROUND], f32, tag="zparts")
                rz = zpool.tile([128, 1], f32, tag="rz")
                vtp = vtpA if pb % 2 == 0 else vtpB
                ets[pb] = et
                vtps[pb] = vtp

                for r in range(NROUND):
                    # keep the PE fed: spread last block's fina waves across
                    # this block's later S rounds (its exp/Z are done by then)
                    if r >= 3 and pb > 0:
                        emit_fina_wave(pb - 1, r - 3)
                    q0 = ROUND * r
                    width = min(ROUND, HW - q0)
                    st = spool.tile([128, ROUND], f32, tag="st")
                    for half in range((width + QCH - 1) // QCH):
                        qo = q0 + QCH * half
                        qw = min(QCH, HW - qo)
                        nc.tensor.matmul(
                            st[:, QCH * half : QCH * half + qw],
                            lhsT=f1s[:, 128 * pb : 128 * pb + 128],
                            rhs=f2s[:, qo : qo + qw],
                            start=True,
                            stop=True,
                        )
                    # bias -5: keeps exp within fp16 range (softmax is
                    # shift-invariant; Z accumulates the same shifted values)
                    nc.scalar.activation(
                        et[:, q0 : q0 + width],
                        st[:, 0:width],
                        EXP,
                        bias=bneg[:, 0:1],
                        accum_out=zparts[:, r : r + 1],
                    )

                # Z = sum of round partials; vtp = vt[:, block] / Z
                nc.vector.tensor_reduce(rz[:, 0:1], zparts[:, :], AXX, ADD)
                nc.vector.reciprocal(rz[:, 0:1], rz[:, 0:1])
                nc.vector.tensor_scalar(
                    vtp[:, 0:NCLS],
                    vts[:, NCLS * pb : NCLS * pb + NCLS],
                    rz[:, 0:1],
                    2048.0,
                    MULT,
                    MULT,
                )

            for t in range(4):
                emit_fina_wave(PBLK - 1, t)
            # live cols are 0..1600 in every partition quadrant (group g's q
            # range is 1600g..1600g+1600); cols 1600..2048 were never written.
            # ScalarE is idle at the tail; copy in halves overlapped with DMA.
            for h in range(2):
                c0, c1 = 800 * h, 800 * h + 800
                nc.scalar.copy(accsb[:, c0:c1], fp[:, c0:c1])
                for g in range(4):
                    nc.sync.dma_start(
                        out=res_d[NCLS * g : NCLS * g + NCLS, c0:c1],
                        in_=accsb[32 * g : 32 * g + NCLS, c0:c1],
                    )

    nc.compile()
    return nc


def _get_nc():
    if "nc" not in _CACHE:
        _CACHE["nc"] = _build_bass()
    return _CACHE["nc"]


def _hilo16(x):
    """fp16 high/low split: x ~= hi + lo exactly to ~2^-22 relative."""
    x = np.asarray(x, np.float32)
    hi = x.astype(np.float16)
    lo = (x - hi.astype(np.float32)).astype(np.float16)
    return hi, lo


def _prep_inputs(feature_in, out, w1, b1, w2, b2):
    feature_in = np.asarray(feature_in, np.float32)
    out = np.asarray(out, np.float32)
    w1 = np.asarray(w1, np.float32)
    b1 = np.asarray(b1, np.float32)
    w2 = np.asarray(w2, np.float32)
    b2 = np.asarray(b2, np.float32)

    scale = np.float32(1.0 / np.sqrt(NCLS))
    feat = feature_in.reshape(NB, C_IN, HW)
    # f1 carries the softmax scale; f2 is plain
    f1 = (np.einsum("oc,ncp->nop", w1, feat, dtype=np.float32) + b1[None, :, None]) * scale
    f2 = np.einsum("oc,ncp->nop", w2, feat, dtype=np.float32) + b2[None, :, None]
    f1 = f1.astype(np.float32)
    f2 = f2.astype(np.float32)
    v = _resize_bilinear_ac(out, H, W).reshape(NB, NCLS, HW)

    in_maps = []
    for core in range(N_CORES):
        b, s = divmod(core, NSH)
        p0 = PSH * s
        f1p = np.zeros((12, PPAD), np.float16)
        h1, l1 = _hilo16(f1[b][:, p0 : p0 + PSH])
        f1p[0:4, :PSH] = h1
        f1p[4:8, :PSH] = l1
        f1p[8:12, :PSH] = h1
        h2, l2 = _hilo16(f2[b])
        f2p = np.concatenate([h2, h2, l2], axis=0)  # [12, HW] fp16
        vtp = np.zeros((NCLS, PPAD), np.float32)
        vtp[:, :PSH] = v[b][:, p0 : p0 + PSH]
        # vt[part, 4*pb + c] = V[c, p0 + 128*pb + part]
        vt = vtp.reshape(NCLS, PBLK, 128).transpose(2, 1, 0).reshape(128, PBLK * NCLS)
        in_maps.append(
            {
                "f1p": f1p,
                "f2p": np.ascontiguousarray(f2p),
                "vt": np.ascontiguousarray(vt),
            }
        )
    return in_maps


def _unpack(results):
    """results: list of 8 dicts with 'res' [16, 2048] -> fina [2,4,80,80]."""
    fina = np.zeros((NB, NCLS, HW), np.float32)
    for core in range(N_CORES):
        b, s = divmod(core, NSH)
        res = np.asarray(results[core]["res"], np.float32)  # [16, 2048]
        part = res.reshape(4, NCLS, 4 * QCH)  # [q-group g, class j, cols]
        for g in range(4):
            fina[b, :, PSH * g : PSH * g + PSH] += part[g][:, :PSH]
    fina *= np.float32(1.0 / 2048.0)
    return fina.reshape(NB, NCLS, H, W)


def run(inputs, trace=False):
    from concourse.bass_utils import run_bass_kernel_spmd

    nc = _get_nc()
    in_maps = _prep_inputs(**inputs)
    r = run_bass_kernel_spmd(nc, in_maps, list(range(N_CORES)), trace=trace)
    return _unpack(r.results), r.exec_time_ns


def kernel(feature_in, out, w1, b1, w2, b2):
    result, _ = run(
        dict(feature_in=feature_in, out=out, w1=w1, b1=b1, w2=w2, b2=b2)
    )
    return result


# revision 9
# speedup vs baseline: 1.0099x; 1.0099x over previous
"""Trainium2 Bass kernel for nn_Corr (correlation-attention module).

Math (per batch n):
    f1 = 0.5*(w1 @ feat + b1)        # [4, 6400]   feat = feature_in[n] flattened
    f2 =      w2 @ feat + b2         # [4, 6400]
    S  = f1^T @ f2                   # [6400, 6400]  (0.5 = 1/sqrt(nclass) folded into f1)
    A  = softmax(S, axis=1)          # row softmax (over q)
    V  = bilinear_resize(out[n])     # [4, 6400]
    fina[c, q] = sum_p V[c, p]/Z_p * exp(S[p, q])

Sharding: 2 batches x 4 p-shards (rows of S) = 8 cores. Each core produces a
partial fina over its 1600 p-rows; host sums the 4 partials per batch.

Device kernel per core (p-shard of 1664 rows incl. pad, all 6400 q):
  - S chunk = matmul(lhsT=f1pad[:, pblock(128 cols)], rhs=f2pad[:, qchunk])
    single M=128 PE tile, N=512 chunks into PSUM round buffers (2x1024 f32).
  - exp on ScalarE PSUM->SBUF (fp16 E strip) with accum_out giving row sums Z.
  - fina = matmul(lhsT=vt*recip(Z), rhs=E) col-tiled 4x (q-group g on PE col
    group g / PSUM partition quadrant g), accumulated IN PSUM across all 13
    p-blocks (start on pb==0, stop on pb==12); one DVE copy + DMA at the end.
  - emit order pipelines fina(pb-1) between S(pb) rounds so the PE never
    waits on the exp of the current block. ScalarE (exp) is the bottleneck
    engine; everything else hides under it.
"""

import numpy as np

N_CORES = 8
NB = 2          # batches
NCLS = 4        # nclass
C_IN = 32
H = W = 80
HW = H * W      # 6400
NSH = 4         # p-shards per batch
PSH = HW // NSH  # 1600 p rows per shard
PBLK = 13        # p blocks of 128 (1664 = 13*128, last 64 rows are zero-pad)
PPAD = PBLK * 128  # 1664
QCH = 512        # q chunk (psum bank)
ROUND = 1024     # q per exp round (2 psum banks)
NROUND = 7       # 6*1024 + 256

_CACHE = {}


def _resize_bilinear_ac(x, h_out, w_out):
    """numpy mirror of the reference's align_corners=True bilinear resize."""
    n, c, h, w = x.shape
    if (h, w) == (h_out, w_out):
        return x
    ys = np.linspace(0.0, h - 1.0, h_out, dtype=np.float32)
    xs = np.linspace(0.0, w - 1.0, w_out, dtype=np.float32)
    y0 = np.floor(ys).astype(np.int32)
    x0 = np.floor(xs).astype(np.int32)
    y1 = np.minimum(y0 + 1, h - 1)
    x1 = np.minimum(x0 + 1, w - 1)
    wy = (ys - y0.astype(np.float32))[None, None, :, None]
    wx = (xs - x0.astype(np.float32))[None, None, None, :]
    g = lambda yi, xi: x[:, :, yi, :][:, :, :, xi]
    top = g(y0, x0) * (1.0 - wx) + g(y0, x1) * wx
    bot = g(y1, x0) * (1.0 - wx) + g(y1, x1) * wx
    return (top * (1.0 - wy) + bot * wy).astype(np.float32)


def _build_bass():
    import concourse.bass as bass
    import concourse.tile as tile
    from concourse import bacc, mybir

    f32 = mybir.dt.float32
    f16 = mybir.dt.float16

    nc = bacc.Bacc(
        "TRN2", target_bir_lowering=False, debug=False, num_devices=N_CORES
    )

    f1p_d = nc.dram_tensor("f1p", [12, PPAD], f16, kind="ExternalInput")
    f2p_d = nc.dram_tensor("f2p", [12, HW], f16, kind="ExternalInput")
    vt_d = nc.dram_tensor("vt", [128, NCLS * PBLK], f32, kind="ExternalInput")
    res_d = nc.dram_tensor("res", [4 * NCLS, 4 * QCH], f32, kind="ExternalOutput")

    EXP = mybir.ActivationFunctionType.Exp
    ADD = mybir.AluOpType.add
    MULT = mybir.AluOpType.mult
    AXX = mybir.AxisListType.X

    with tile.TileContext(nc) as tc:
        with (
            tc.tile_pool(name="const", bufs=1) as cpool,
            tc.tile_pool(name="estrip", bufs=2) as epool,
            tc.tile_pool(name="zpool", bufs=2) as zpool,
            tc.tile_pool(name="spsum", bufs=2, space="PSUM") as spool,
            tc.tile_pool(name="fpsum", bufs=1, space="PSUM") as fpool,
        ):
            # K=128 keeps FWL (fast weight load) active; pad rows 12..127 are
            # zeroed on the DVE below, hidden under the input DMAs
            f1s = cpool.tile([128, PPAD], f16, tag="f1s")
            f2s = cpool.tile([128, HW], f16, tag="f2s")
            vts = cpool.tile([128, NCLS * PBLK], f32, tag="vts")
            accsb = cpool.tile([128, 4 * QCH], f32, tag="accsb")
            # ping-pong [128, 32] lhsT tiles for fina (cols 4..31 stay zero)
            vtpA = cpool.tile([128, 32], f16, tag="vtpA")
            vtpB = cpool.tile([128, 32], f16, tag="vtpB")
            bneg = cpool.tile([128, 1], f32, tag="bneg")
            scr = cpool.tile([128, 1], f32, tag="scr")
            # persistent fina accumulator: 4 PSUM banks, accumulated over all
            # p-blocks by the PE itself (start only on the first block)
            fp = fpool.tile([128, 4 * QCH], f32, tag="fp")

            nc.gpsimd.memset(vtpA[:, :], 0.0)
            nc.gpsimd.memset(vtpB[:, :], 0.0)
            nc.gpsimd.memset(bneg[:, :], -5.0)
            nc.vector.memset(f1s[:, :], 0.0)
            nc.vector.memset(f2s[:, :], 0.0)
            # dummy exp so the ~2.7us ACT table load overlaps the input DMAs
            nc.scalar.activation(scr[:, 0:1], bneg[:, 0:1], EXP)
            nc.sync.dma_start(out=f1s[0:12, :], in_=f1p_d[:, :])
            nc.sync.dma_start(out=f2s[0:12, :], in_=f2p_d[:, :])
            nc.sync.dma_start(out=vts[:, :], in_=vt_d[:, :])

            ets = [None] * PBLK
            vtps = [None] * PBLK

            def emit_fina_wave(pb, t):
                # col-tiled 4x: group g covers q in [1600g, 1600g+1600) as
                # waves of N=512,512,512,64; PSUM bank t cols 512t..; group g
                # on partition quadrant 32g. Accumulate across p-blocks.
                et = ets[pb]
                vtp = vtps[pb]
                qw = 512 if t < 3 else 64
                for g in range(4):
                    qo = 1600 * g + QCH * t
                    nc.tensor.matmul(
                        fp[32 * g : 32 * g + 32, QCH * t : QCH * t + qw],
                        lhsT=vtp[:, :],
                        rhs=et[:, qo : qo + qw],
                        start=(pb == 0),
                        stop=(pb == PBLK - 1),
                        tile_position=(0, 32 * g),
                        skip_group_check=True,
                    )

            for pb in range(PBLK):
                et = epool.tile([128, HW], f16, tag="et")
                zparts = zpool.tile([128, NROUND], f32, tag="zparts")
                rz = zpool.tile([128, 1], f32, tag="rz")
                vtp = vtpA if pb % 2 == 0 else vtpB
                ets[pb] = et
                vtps[pb] = vtp

                for r in range(NROUND):
                    # keep the PE fed: spread last block's fina waves across
                    # this block's later S rounds (its exp/Z are done by then)
                    if r >= 3 and pb > 0:
                        emit_fina_wave(pb - 1, r - 3)
                    q0 = ROUND * r
                    width = min(ROUND, HW - q0)
                    st = spool.tile([128, ROUND], f32, tag="st")
                    for half in range((width + QCH - 1) // QCH):
                        qo = q0 + QCH * half
                        qw = min(QCH, HW - qo)
                        nc.tensor.matmul(
                            st[:, QCH * half : QCH * half + qw],
                            lhsT=f1s[:, 128 * pb : 128 * pb + 128],
                            rhs=f2s[:, qo : qo + qw],
                            start=True,
                            stop=True,
                        )
                    # bias -5: keeps exp within fp16 range (softmax is
                    # shift-invariant; Z accumulates the same shifted values)
                    nc.scalar.activation(
                        et[:, q0 : q0 + width],
                        st[:, 0:width],
                        EXP,
                        bias=bneg[:, 0:1],
                        accum_out=zparts[:, r : r + 1],
                    )

                # Z = sum of round partials; vtp = vt[:, block] / Z
                nc.vector.tensor_reduce(rz[:, 0:1], zparts[:, :], AXX, ADD)
                nc.vector.reciprocal(rz[:, 0:1], rz[:, 0:1])
                nc.vector.tensor_scalar(
                    vtp[:, 0:NCLS],
                    vts[:, NCLS * pb : NCLS * pb + NCLS],
                    rz[:, 0:1],
                    2048.0,
                    MULT,
                    MULT,
                )

            for t in range(4):
                emit_fina_wave(PBLK - 1, t)
            # live cols are 0..1600 in every partition quadrant (group g's q
            # range is 1600g..1600g+1600); cols 1600..2048 were never written.
            # ScalarE is idle at the tail; copy in halves overlapped with DMA.
            for h in range(2):
                c0, c1 = 800 * h, 800 * h + 800
                nc.scalar.copy(accsb[:, c0:c1], fp[:, c0:c1])
                for g in range(4):
                    nc.sync.dma_start(
                        out=res_d[NCLS * g : NCLS * g + NCLS, c0:c1],
                        in_=accsb[32 * g : 32 * g + NCLS, c0:c1],
                    )

    nc.compile()
    return nc


def _get_nc():
    if "nc" not in _CACHE:
        _CACHE["nc"] = _build_bass()
    return _CACHE["nc"]


def _hilo16(x):
    """fp16 high/low split: x ~= hi + lo exactly to ~2^-22 relative."""
    x = np.asarray(x, np.float32)
    hi = x.astype(np.float16)
    lo = (x - hi.astype(np.float32)).astype(np.float16)
    return hi, lo


def _prep_inputs(feature_in, out, w1, b1, w2, b2):
    feature_in = np.asarray(feature_in, np.float32)
    out = np.asarray(out, np.float32)
    w1 = np.asarray(w1, np.float32)
    b1 = np.asarray(b1, np.float32)
    w2 = np.asarray(w2, np.float32)
    b2 = np.asarray(b2, np.float32)

    scale = np.float32(1.0 / np.sqrt(NCLS))
    feat = feature_in.reshape(NB, C_IN, HW)
    # f1 carries the softmax scale; f2 is plain
    f1 = (np.einsum("oc,ncp->nop", w1, feat, dtype=np.float32) + b1[None, :, None]) * scale
    f2 = np.einsum("oc,ncp->nop", w2, feat, dtype=np.float32) + b2[None, :, None]
    f1 = f1.astype(np.float32)
    f2 = f2.astype(np.float32)
    v = _resize_bilinear_ac(out, H, W).reshape(NB, NCLS, HW)

    in_maps = []
    for core in range(N_CORES):
        b, s = divmod(core, NSH)
        p0 = PSH * s
        f1p = np.zeros((12, PPAD), np.float16)
        h1, l1 = _hilo16(f1[b][:, p0 : p0 + PSH])
        f1p[0:4, :PSH] = h1
        f1p[4:8, :PSH] = l1
        f1p[8:12, :PSH] = h1
        h2, l2 = _hilo16(f2[b])
        f2p = np.concatenate([h2, h2, l2], axis=0)  # [12, HW] fp16
        vtp = np.zeros((NCLS, PPAD), np.float32)
        vtp[:, :PSH] = v[b][:, p0 : p0 + PSH]
        # vt[part, 4*pb + c] = V[c, p0 + 128*pb + part]
        vt = vtp.reshape(NCLS, PBLK, 128).transpose(2, 1, 0).reshape(128, PBLK * NCLS)
        in_maps.append(
            {
                "f1p": f1p,
                "f2p": np.ascontiguousarray(f2p),
                "vt": np.ascontiguousarray(vt),
            }
        )
    return in_maps


def _unpack(results):
    """results: list of 8 dicts with 'res' [16, 2048] -> fina [2,4,80,80]."""
    fina = np.zeros((NB, NCLS, HW), np.float32)
    for core in range(N_CORES):
        b, s = divmod(core, NSH)
        res = np.asarray(results[core]["res"], np.float32)  # [16, 2048]
        part = res.reshape(4, NCLS, 4 * QCH)  # [q-group g, class j, cols]
        for g in range(4):
            fina[b, :, PSH * g : PSH * g + PSH] += part[g][:, :PSH]
    fina *= np.float32(1.0 / 2048.0)
    return fina.reshape(NB, NCLS, H, W)


def run(inputs, trace=False):
    from concourse.bass_utils import run_bass_kernel_spmd

    nc = _get_nc()
    in_maps = _prep_inputs(**inputs)
    r = run_bass_kernel_spmd(nc, in_maps, list(range(N_CORES)), trace=trace)
    return _unpack(r.results), r.exec_time_ns


def kernel(feature_in, out, w1, b1, w2, b2):
    result, _ = run(
        dict(feature_in=feature_in, out=out, w1=w1, b1=b1, w2=w2, b2=b2)
    )
    return result


# revision 12
# speedup vs baseline: 1.1158x; 1.1048x over previous
"""Trainium2 Bass kernel for nn_Corr (correlation-attention module).

Math (per batch n):
    f1 = 0.5*(w1 @ feat + b1)        # [4, 6400]   feat = feature_in[n] flattened
    f2 =      w2 @ feat + b2         # [4, 6400]
    S  = f1^T @ f2                   # [6400, 6400]  (0.5 = 1/sqrt(nclass) folded into f1)
    A  = softmax(S, axis=1)          # row softmax (over q)
    V  = bilinear_resize(out[n])     # [4, 6400]
    fina[c, q] = sum_p V[c, p]/Z_p * exp(S[p, q])

Sharding: 2 batches x 4 p-shards (rows of S) = 8 cores. Each core produces a
partial fina over its 1600 p-rows; host sums the 4 partials per batch.

Device kernel per core (p-shard of 1664 rows incl. pad, all 6400 q).
ScalarE (exp, 1 elem/lane/cycle) is the bottleneck engine; the structure
minimizes ScalarE instruction count and hides everything else under it:
  - S chunk = matmul(lhsT=f1pad[:, pblock(128 cols)], rhs=f2pad[:, qchunk])
    single M=128 PE tile (K=128 keeps fast-weight-load), N=512 chunks into
    PSUM round buffers of 1536 f32 (3 banks, double buffered = 6 banks).
  - exp on ScalarE PSUM->SBUF (fp16 E strip) in 5 rounds/block
    (4x1536 + 256) with accum_out giving row sums Z.
  - fina = matmul(lhsT=vt*recip(Z), rhs=E) col-tiled 4x (q-group g on PE col
    group g) into a 1-bank PSUM scratch (double buffered = 2 banks), then
    DVE-accumulated into an SBUF accumulator, one 512-wide wave per S round
    of the next block so the PE and DVE loads stay smooth.
"""

import numpy as np

N_CORES = 8
NB = 2          # batches
NCLS = 4        # nclass
C_IN = 32
H = W = 80
HW = H * W      # 6400
NSH = 4         # p-shards per batch
PSH = HW // NSH  # 1600 p rows per shard
PBLK = 13        # p blocks of 128 (1664 = 13*128, last 64 rows are zero-pad)
PPAD = PBLK * 128  # 1664
QCH = 512        # q chunk (psum bank)
ROUND = 1536     # q per exp round (3 psum banks)
RWID = (1536, 1536, 1536, 1536, 256)
NROUND = 5

_CACHE = {}


def _resize_bilinear_ac(x, h_out, w_out):
    """numpy mirror of the reference's align_corners=True bilinear resize."""
    n, c, h, w = x.shape
    if (h, w) == (h_out, w_out):
        return x
    ys = np.linspace(0.0, h - 1.0, h_out, dtype=np.float32)
    xs = np.linspace(0.0, w - 1.0, w_out, dtype=np.float32)
    y0 = np.floor(ys).astype(np.int32)
    x0 = np.floor(xs).astype(np.int32)
    y1 = np.minimum(y0 + 1, h - 1)
    x1 = np.minimum(x0 + 1, w - 1)
    wy = (ys - y0.astype(np.float32))[None, None, :, None]
    wx = (xs - x0.astype(np.float32))[None, None, None, :]
    g = lambda yi, xi: x[:, :, yi, :][:, :, :, xi]
    top = g(y0, x0) * (1.0 - wx) + g(y0, x1) * wx
    bot = g(y1, x0) * (1.0 - wx) + g(y1, x1) * wx
    return (top * (1.0 - wy) + bot * wy).astype(np.float32)


def _build_bass():
    import concourse.bass as bass
    import concourse.tile as tile
    from concourse import bacc, mybir

    f32 = mybir.dt.float32
    f16 = mybir.dt.float16
    u32 = mybir.dt.uint32

    nc = bacc.Bacc(
        "TRN2", target_bir_lowering=False, debug=False, num_devices=N_CORES
    )

    f1p_d = nc.dram_tensor("f1p", [12, PPAD], f16, kind="ExternalInput")
    f2p_d = nc.dram_tensor("f2p", [12, HW], f16, kind="ExternalInput")
    vt_d = nc.dram_tensor("vt", [128, NCLS * PBLK], f32, kind="ExternalInput")
    res_d = nc.dram_tensor("res", [4 * NCLS, 4 * QCH], f32, kind="ExternalOutput")

    EXP = mybir.ActivationFunctionType.Exp
    ADD = mybir.AluOpType.add
    MULT = mybir.AluOpType.mult
    AXX = mybir.AxisListType.X

    with tile.TileContext(nc) as tc:
        with (
            tc.tile_pool(name="const", bufs=1) as cpool,
            tc.tile_pool(name="estrip", bufs=2) as epool,
            tc.tile_pool(name="zpool", bufs=2) as zpool,
            tc.tile_pool(name="spsum", bufs=2, space="PSUM") as spool,
            tc.tile_pool(name="fpsum", bufs=2, space="PSUM") as fpool,
        ):
            # K=128 keeps FWL (fast weight load) active; pad rows 12..127 are
            # zeroed below (u32-bitcast memsets, split across DVE and GpSimd,
            # hidden under the kernel preamble + input DMAs)
            f1s = cpool.tile([128, PPAD], f16, tag="f1s")
            f2s = cpool.tile([128, HW], f16, tag="f2s")
            vts = cpool.tile([128, NCLS * PBLK], f32, tag="vts")
            accsb = cpool.tile([128, 4 * QCH], f32, tag="accsb")
            # ping-pong [128, 32] lhsT tiles for fina (cols 4..31 stay zero)
            vtpA = cpool.tile([128, 32], f16, tag="vtpA")
            vtpB = cpool.tile([128, 32], f16, tag="vtpB")
            bneg = cpool.tile([128, 1], f32, tag="bneg")
            scr = cpool.tile([128, 1], f32, tag="scr")

            nc.gpsimd.memset(vtpA[:, :], 0.0)
            nc.gpsimd.memset(vtpB[:, :], 0.0)
            nc.gpsimd.memset(bneg[:, :], -5.0)
            nc.gpsimd.memset(f1s[:, :].bitcast(u32), 0)
            nc.vector.memset(f2s[:, :].bitcast(u32), 0)
            # dummy exp so the ~2.7us ACT table load overlaps the input DMAs
            nc.scalar.activation(scr[:, 0:1], bneg[:, 0:1], EXP)
            nc.sync.dma_start(out=f1s[0:12, :], in_=f1p_d[:, :])
            nc.sync.dma_start(out=f2s[0:12, :], in_=f2p_d[:, :])
            nc.sync.dma_start(out=vts[:, :], in_=vt_d[:, :])

            ets = [None] * PBLK
            vtps = [None] * PBLK

            def emit_fina_wave(pb, t):
                # col-tiled 4x: group g covers q in [1600g, 1600g+1600) as
                # waves of N=512,512,512,64 into a 1-bank PSUM scratch;
                # group g on partition quadrant 32g. DVE accumulates the
                # scratch into the SBUF accumulator (cols 512t..512t+qw).
                et = ets[pb]
                vtp = vtps[pb]
                qw = 512 if t < 3 else 64
                fsc = fpool.tile([128, QCH], f32, tag="fsc")
                for g in range(4):
                    qo = 1600 * g + QCH * t
                    nc.tensor.matmul(
                        fsc[32 * g : 32 * g + 32, 0:qw],
                        lhsT=vtp[:, :],
                        rhs=et[:, qo : qo + qw],
                        start=True,
                        stop=True,
                        tile_position=(0, 32 * g),
                    )
                dst = accsb[:, QCH * t : QCH * t + qw]
                if pb == 0:
                    nc.vector.tensor_copy(dst, fsc[:, 0:qw])
                else:
                    nc.vector.tensor_add(dst, dst, fsc[:, 0:qw])

            for pb in range(PBLK):
                et = epool.tile([128, HW], f16, tag="et")
                zparts = zpool.tile([128, NROUND], f32, tag="zparts")
                rz = zpool.tile([128, 1], f32, tag="rz")
                vtp = vtpA if pb % 2 == 0 else vtpB
                ets[pb] = et
                vtps[pb] = vtp

                q0 = 0
                for r in range(NROUND):
                    # keep the PE/DVE fed: spread last block's fina waves
                    # across this block's later S rounds
                    if r >= 1 and pb > 0:
                        emit_fina_wave(pb - 1, r - 1)
                    width = RWID[r]
                    st = spool.tile([128, ROUND], f32, tag="st")
                    for half in range((width + QCH - 1) // QCH):
                        qo = q0 + QCH * half
                        qw = min(QCH, width - QCH * half)
                        nc.tensor.matmul(
                            st[:, QCH * half : QCH * half + qw],
                            lhsT=f1s[:, 128 * pb : 128 * pb + 128],
                            rhs=f2s[:, qo : qo + qw],
                            start=True,
                            stop=True,
                        )
                    # bias -5: keeps exp within fp16 range (softmax is
                    # shift-invariant; Z accumulates the same shifted values)
                    nc.scalar.activation(
                        et[:, q0 : q0 + width],
                        st[:, 0:width],
                        EXP,
                        bias=bneg[:, 0:1],
                        accum_out=zparts[:, r : r + 1],
                    )
                    q0 += width

                # Z = sum of round partials; vtp = vt[:, block] / Z
                nc.vector.tensor_reduce(rz[:, 0:1], zparts[:, :], AXX, ADD)
                nc.vector.reciprocal(rz[:, 0:1], rz[:, 0:1])
                nc.vector.tensor_scalar(
                    vtp[:, 0:NCLS],
                    vts[:, NCLS * pb : NCLS * pb + NCLS],
                    rz[:, 0:1],
                    2048.0,
                    MULT,
                    MULT,
                )

            for t in range(4):
                emit_fina_wave(PBLK - 1, t)
            # result is already in SBUF; issue the 4 group DMAs from four
            # different engine queues so their descriptor setup overlaps
            dma_engines = [nc.sync, nc.scalar, nc.gpsimd, nc.sync]
            for g in range(4):
                dma_engines[g].dma_start(
                    out=res_d[NCLS * g : NCLS * g + NCLS, 0:PSH],
                    in_=accsb[32 * g : 32 * g + NCLS, 0:PSH],
                )

    nc.compile()
    return nc


def _get_nc():
    if "nc" not in _CACHE:
        _CACHE["nc"] = _build_bass()
    return _CACHE["nc"]


def _hilo16(x):
    """fp16 high/low split: x ~= hi + lo exactly to ~2^-22 relative."""
    x = np.asarray(x, np.float32)
    hi = x.astype(np.float16)
    lo = (x - hi.astype(np.float32)).astype(np.float16)
    return hi, lo


def _prep_inputs(feature_in, out, w1, b1, w2, b2):
    feature_in = np.asarray(feature_in, np.float32)
    out = np.asarray(out, np.float32)
    w1 = np.asarray(w1, np.float32)
    b1 = np.asarray(b1, np.float32)
    w2 = np.asarray(w2, np.float32)
    b2 = np.asarray(b2, np.float32)

    scale = np.float32(1.0 / np.sqrt(NCLS))
    feat = feature_in.reshape(NB, C_IN, HW)
    # f1 carries the softmax scale; f2 is plain
    f1 = (np.einsum("oc,ncp->nop", w1, feat, dtype=np.float32) + b1[None, :, None]) * scale
    f2 = np.einsum("oc,ncp->nop", w2, feat, dtype=np.float32) + b2[None, :, None]
    f1 = f1.astype(np.float32)
    f2 = f2.astype(np.float32)
    v = _resize_bilinear_ac(out, H, W).reshape(NB, NCLS, HW)

    in_maps = []
    for core in range(N_CORES):
        b, s = divmod(core, NSH)
        p0 = PSH * s
        f1p = np.zeros((12, PPAD), np.float16)
        h1, l1 = _hilo16(f1[b][:, p0 : p0 + PSH])
        f1p[0:4, :PSH] = h1
        f1p[4:8, :PSH] = l1
        f1p[8:12, :PSH] = h1
        h2, l2 = _hilo16(f2[b])
        f2p = np.concatenate([h2, h2, l2], axis=0)  # [12, HW] fp16
        vtp = np.zeros((NCLS, PPAD), np.float32)
        vtp[:, :PSH] = v[b][:, p0 : p0 + PSH]
        # vt[part, 4*pb + c] = V[c, p0 + 128*pb + part]
        vt = vtp.reshape(NCLS, PBLK, 128).transpose(2, 1, 0).reshape(128, PBLK * NCLS)
        in_maps.append(
            {
                "f1p": f1p,
                "f2p": np.ascontiguousarray(f2p),
                "vt": np.ascontiguousarray(vt),
            }
        )
    return in_maps


def _unpack(results):
    """results: list of 8 dicts with 'res' [16, 2048] -> fina [2,4,80,80]."""
    fina = np.zeros((NB, NCLS, HW), np.float32)
    for core in range(N_CORES):
        b, s = divmod(core, NSH)
        res = np.asarray(results[core]["res"], np.float32)  # [16, 2048]
        part = res.reshape(4, NCLS, 4 * QCH)  # [q-group g, class j, cols]
        for g in range(4):
            fina[b, :, PSH * g : PSH * g + PSH] += part[g][:, :PSH]
    fina *= np.float32(1.0 / 2048.0)
    return fina.reshape(NB, NCLS, H, W)


def run(inputs, trace=False):
    from concourse.bass_utils import run_bass_kernel_spmd

    nc = _get_nc()
    in_maps = _prep_inputs(**inputs)
    r = run_bass_kernel_spmd(nc, in_maps, list(range(N_CORES)), trace=trace)
    return _unpack(r.results), r.exec_time_ns


def kernel(feature_in, out, w1, b1, w2, b2):
    result, _ = run(
        dict(feature_in=feature_in, out=out, w1=w1, b1=b1, w2=w2, b2=b2)
    )
    return result


# revision 15
# speedup vs baseline: 1.1309x; 1.0136x over previous
"""Trainium2 Bass kernel for nn_Corr (correlation-attention module).

Math (per batch n):
    f1 = 0.5*(w1 @ feat + b1)        # [4, 6400]   feat = feature_in[n] flattened
    f2 =      w2 @ feat + b2         # [4, 6400]
    S  = f1^T @ f2                   # [6400, 6400]  (0.5 = 1/sqrt(nclass) folded into f1)
    A  = softmax(S, axis=1)          # row softmax (over q)
    V  = bilinear_resize(out[n])     # [4, 6400]
    fina[c, q] = sum_p V[c, p]/Z_p * exp(S[p, q])

Sharding: 2 batches x 4 p-shards (rows of S) = 8 cores. Each core produces a
partial fina over its 1600 p-rows; host sums the 4 partials per batch.

Device kernel per core (p-shard of 1664 rows incl. pad, all 6400 q).
ScalarE (exp, 1 elem/lane/cycle) is the bottleneck engine; the structure
minimizes ScalarE instruction count and hides everything else under it:
  - S chunk = matmul(lhsT=f1pad[:, pblock(128 cols)], rhs=f2pad[:, qchunk])
    single M=128 PE tile (K=128 keeps fast-weight-load), N=512 chunks into
    PSUM round buffers of 1536 f32 (3 banks, double buffered = 6 banks).
  - exp on ScalarE PSUM->SBUF (fp16 E strip) in 5 rounds/block
    (4x1536 + 256) with accum_out giving row sums Z.
  - fina = matmul(lhsT=vt*recip(Z), rhs=E) col-tiled 4x (q-group g on PE col
    group g) into a 1-bank PSUM scratch (double buffered = 2 banks), then
    DVE-accumulated into an SBUF accumulator, one 512-wide wave per S round
    of the next block so the PE and DVE loads stay smooth.
"""

import numpy as np

N_CORES = 8
NB = 2          # batches
NCLS = 4        # nclass
C_IN = 32
H = W = 80
HW = H * W      # 6400
NSH = 4         # p-shards per batch
PSH = HW // NSH  # 1600 p rows per shard
PBLK = 13        # p blocks of 128 (1664 = 13*128, last 64 rows are zero-pad)
PPAD = PBLK * 128  # 1664
QCH = 512        # q chunk (psum bank)
ROUND = 1536     # q per exp round (3 psum banks)
RWID = (1536, 1536, 1536, 1536, 256)
NROUND = 5

_CACHE = {}


def _resize_bilinear_ac(x, h_out, w_out):
    """numpy mirror of the reference's align_corners=True bilinear resize."""
    n, c, h, w = x.shape
    if (h, w) == (h_out, w_out):
        return x
    ys = np.linspace(0.0, h - 1.0, h_out, dtype=np.float32)
    xs = np.linspace(0.0, w - 1.0, w_out, dtype=np.float32)
    y0 = np.floor(ys).astype(np.int32)
    x0 = np.floor(xs).astype(np.int32)
    y1 = np.minimum(y0 + 1, h - 1)
    x1 = np.minimum(x0 + 1, w - 1)
    wy = (ys - y0.astype(np.float32))[None, None, :, None]
    wx = (xs - x0.astype(np.float32))[None, None, None, :]
    g = lambda yi, xi: x[:, :, yi, :][:, :, :, xi]
    top = g(y0, x0) * (1.0 - wx) + g(y0, x1) * wx
    bot = g(y1, x0) * (1.0 - wx) + g(y1, x1) * wx
    return (top * (1.0 - wy) + bot * wy).astype(np.float32)


def _build_bass():
    import concourse.bass as bass
    import concourse.tile as tile
    from concourse import bacc, mybir

    f32 = mybir.dt.float32
    f16 = mybir.dt.float16
    u32 = mybir.dt.uint32

    nc = bacc.Bacc(
        "TRN2", target_bir_lowering=False, debug=False, num_devices=N_CORES
    )

    f1p_d = nc.dram_tensor("f1p", [12, PPAD], f16, kind="ExternalInput")
    f2p_d = nc.dram_tensor("f2p", [12, HW], f16, kind="ExternalInput")
    vt_d = nc.dram_tensor("vt", [128, NCLS * PBLK], f32, kind="ExternalInput")
    res_d = nc.dram_tensor("res", [4 * NCLS, 4 * QCH], f32, kind="ExternalOutput")

    EXP = mybir.ActivationFunctionType.Exp
    ADD = mybir.AluOpType.add
    MULT = mybir.AluOpType.mult
    AXX = mybir.AxisListType.X

    with tile.TileContext(nc) as tc:
        with (
            tc.tile_pool(name="const", bufs=1) as cpool,
            tc.tile_pool(name="estrip", bufs=2) as epool,
            tc.tile_pool(name="zpool", bufs=2) as zpool,
            tc.tile_pool(name="spsum", bufs=2, space="PSUM") as spool,
            tc.tile_pool(name="fpsum", bufs=2, space="PSUM") as fpool,
        ):
            # K=128 keeps FWL (fast weight load) active; pad rows 12..127 are
            # zeroed below (u32-bitcast memsets, split across DVE and GpSimd,
            # hidden under the kernel preamble + input DMAs)
            f1s = cpool.tile([128, PPAD], f16, tag="f1s")
            f2s = cpool.tile([128, HW], f16, tag="f2s")
            vts = cpool.tile([128, NCLS * PBLK], f32, tag="vts")
            accsb = cpool.tile([128, 4 * QCH], f32, tag="accsb")
            # ping-pong [128, 32] lhsT tiles for fina (cols 4..31 stay zero)
            vtpA = cpool.tile([128, 32], f16, tag="vtpA")
            vtpB = cpool.tile([128, 32], f16, tag="vtpB")
            bneg = cpool.tile([128, 1], f32, tag="bneg")
            scr = cpool.tile([128, 1], f32, tag="scr")

            # pad-row zeroing split across GpSimd and DVE so it finishes in
            # ~2us; the ACT table load (dummy exp) overlaps it on ScalarE
            nc.gpsimd.memset(vtpA[:, :], 0.0)
            nc.gpsimd.memset(vtpB[:, :], 0.0)
            nc.gpsimd.memset(bneg[:, :], -5.0)
            nc.gpsimd.memset(f2s[:, HW // 2 :].bitcast(u32), 0)
            nc.vector.memset(f1s[:, :].bitcast(u32), 0)
            nc.vector.memset(f2s[:, 0 : HW // 2].bitcast(u32), 0)
            nc.scalar.activation(scr[:, 0:1], bneg[:, 0:1], EXP)
            # input DMAs: first S round needs f2 cols 0:1536 and f1 only
            nc.sync.dma_start(out=f2s[0:12, 0:ROUND], in_=f2p_d[:, 0:ROUND])
            nc.sync.dma_start(out=f1s[0:12, :], in_=f1p_d[:, :])
            nc.sync.dma_start(out=f2s[0:12, ROUND:], in_=f2p_d[:, ROUND:])
            nc.sync.dma_start(out=vts[:, :], in_=vt_d[:, :])

            ets = [None] * PBLK
            vtps = [None] * PBLK

            def emit_fina_wave(pb, t):
                # col-tiled 4x: group g covers q in [1600g, 1600g+1600) as
                # waves of N=512,512,512,64 into a 1-bank PSUM scratch;
                # group g on partition quadrant 32g. DVE accumulates the
                # scratch into the SBUF accumulator (cols 512t..512t+qw).
                et = ets[pb]
                vtp = vtps[pb]
                qw = 512 if t < 3 else 64
                fsc = fpool.tile([128, QCH], f32, tag="fsc")
                for g in range(4):
                    qo = 1600 * g + QCH * t
                    nc.tensor.matmul(
                        fsc[32 * g : 32 * g + 32, 0:qw],
                        lhsT=vtp[:, :],
                        rhs=et[:, qo : qo + qw],
                        start=True,
                        stop=True,
                        tile_position=(0, 32 * g),
                    )
                dst = accsb[:, QCH * t : QCH * t + qw]
                if pb == 0:
                    nc.vector.tensor_copy(dst, fsc[:, 0:qw])
                else:
                    nc.vector.tensor_add(dst, dst, fsc[:, 0:qw])

            for pb in range(PBLK):
                et = epool.tile([128, HW], f16, tag="et")
                zparts = zpool.tile([128, NROUND], f32, tag="zparts")
                rz = zpool.tile([128, 1], f32, tag="rz")
                vtp = vtpA if pb % 2 == 0 else vtpB
                ets[pb] = et
                vtps[pb] = vtp

                q0 = 0
                for r in range(NROUND):
                    # keep the PE/DVE fed: spread last block's fina waves
                    # across this block's later S rounds
                    if r >= 1 and pb > 0:
                        emit_fina_wave(pb - 1, r - 1)
                    width = RWID[r]
                    st = spool.tile([128, ROUND], f32, tag="st")
                    for half in range((width + QCH - 1) // QCH):
                        qo = q0 + QCH * half
                        qw = min(QCH, width - QCH * half)
                        nc.tensor.matmul(
                            st[:, QCH * half : QCH * half + qw],
                            lhsT=f1s[:, 128 * pb : 128 * pb + 128],
                            rhs=f2s[:, qo : qo + qw],
                            start=True,
                            stop=True,
                        )
                    # bias -5: keeps exp within fp16 range (softmax is
                    # shift-invariant; Z accumulates the same shifted values)
                    nc.scalar.activation(
                        et[:, q0 : q0 + width],
                        st[:, 0:width],
                        EXP,
                        bias=bneg[:, 0:1],
                        accum_out=zparts[:, r : r + 1],
                    )
                    q0 += width

                # Z = sum of round partials; vtp = vt[:, block] / Z
                nc.vector.tensor_reduce(rz[:, 0:1], zparts[:, :], AXX, ADD)
                nc.vector.reciprocal(rz[:, 0:1], rz[:, 0:1])
                nc.vector.tensor_scalar(
                    vtp[:, 0:NCLS],
                    vts[:, NCLS * pb : NCLS * pb + NCLS],
                    rz[:, 0:1],
                    2048.0,
                    MULT,
                    MULT,
                )

            # result lands in SBUF as the last block's waves complete; cols
            # 0:1536 are final after wave 2's add, so their DMAs are emitted
            # before wave 3 to overlap it. Issues spread over the
            # sync/scalar/gpsimd queues so descriptor setup overlaps.
            dma_engines = [nc.sync, nc.scalar, nc.gpsimd, nc.sync]
            for t in range(3):
                emit_fina_wave(PBLK - 1, t)
            for g in range(4):
                dma_engines[g].dma_start(
                    out=res_d[NCLS * g : NCLS * g + NCLS, 0 : 3 * QCH],
                    in_=accsb[32 * g : 32 * g + NCLS, 0 : 3 * QCH],
                )
            emit_fina_wave(PBLK - 1, 3)
            for g in range(4):
                dma_engines[(g + 1) % 3].dma_start(
                    out=res_d[NCLS * g : NCLS * g + NCLS, 3 * QCH : PSH],
                    in_=accsb[32 * g : 32 * g + NCLS, 3 * QCH : PSH],
                )

    nc.compile()
    return nc


def _get_nc():
    if "nc" not in _CACHE:
        _CACHE["nc"] = _build_bass()
    return _CACHE["nc"]


def _hilo16(x):
    """fp16 high/low split: x ~= hi + lo exactly to ~2^-22 relative."""
    x = np.asarray(x, np.float32)
    hi = x.astype(np.float16)
    lo = (x - hi.astype(np.float32)).astype(np.float16)
    return hi, lo


def _prep_inputs(feature_in, out, w1, b1, w2, b2):
    feature_in = np.asarray(feature_in, np.float32)
    out = np.asarray(out, np.float32)
    w1 = np.asarray(w1, np.float32)
    b1 = np.asarray(b1, np.float32)
    w2 = np.asarray(w2, np.float32)
    b2 = np.asarray(b2, np.float32)

    scale = np.float32(1.0 / np.sqrt(NCLS))
    feat = feature_in.reshape(NB, C_IN, HW)
    # f1 carries the softmax scale; f2 is plain
    f1 = (np.einsum("oc,ncp->nop", w1, feat, dtype=np.float32) + b1[None, :, None]) * scale
    f2 = np.einsum("oc,ncp->nop", w2, feat, dtype=np.float32) + b2[None, :, None]
    f1 = f1.astype(np.float32)
    f2 = f2.astype(np.float32)
    v = _resize_bilinear_ac(out, H, W).reshape(NB, NCLS, HW)

    in_maps = []
    for core in range(N_CORES):
        b, s = divmod(core, NSH)
        p0 = PSH * s
        f1p = np.zeros((12, PPAD), np.float16)
        h1, l1 = _hilo16(f1[b][:, p0 : p0 + PSH])
        f1p[0:4, :PSH] = h1
        f1p[4:8, :PSH] = l1
        f1p[8:12, :PSH] = h1
        h2, l2 = _hilo16(f2[b])
        f2p = np.concatenate([h2, h2, l2], axis=0)  # [12, HW] fp16
        vtp = np.zeros((NCLS, PPAD), np.float32)
        vtp[:, :PSH] = v[b][:, p0 : p0 + PSH]
        # vt[part, 4*pb + c] = V[c, p0 + 128*pb + part]
        vt = vtp.reshape(NCLS, PBLK, 128).transpose(2, 1, 0).reshape(128, PBLK * NCLS)
        in_maps.append(
            {
                "f1p": f1p,
                "f2p": np.ascontiguousarray(f2p),
                "vt": np.ascontiguousarray(vt),
            }
        )
    return in_maps


def _unpack(results):
    """results: list of 8 dicts with 'res' [16, 2048] -> fina [2,4,80,80]."""
    fina = np.zeros((NB, NCLS, HW), np.float32)
    for core in range(N_CORES):
        b, s = divmod(core, NSH)
        res = np.asarray(results[core]["res"], np.float32)  # [16, 2048]
        part = res.reshape(4, NCLS, 4 * QCH)  # [q-group g, class j, cols]
        for g in range(4):
            fina[b, :, PSH * g : PSH * g + PSH] += part[g][:, :PSH]
    fina *= np.float32(1.0 / 2048.0)
    return fina.reshape(NB, NCLS, H, W)


def run(inputs, trace=False):
    from concourse.bass_utils import run_bass_kernel_spmd

    nc = _get_nc()
    in_maps = _prep_inputs(**inputs)
    r = run_bass_kernel_spmd(nc, in_maps, list(range(N_CORES)), trace=trace)
    return _unpack(r.results), r.exec_time_ns


def kernel(feature_in, out, w1, b1, w2, b2):
    result, _ = run(
        dict(feature_in=feature_in, out=out, w1=w1, b1=b1, w2=w2, b2=b2)
    )
    return result


# revision 18
# speedup vs baseline: 1.2112x; 1.0710x over previous
"""Trainium2 Bass kernel for nn_Corr (correlation-attention module).

Math (per batch n):
    f1 = 0.5*(w1 @ feat + b1)        # [4, 6400]   feat = feature_in[n] flattened
    f2 =      w2 @ feat + b2         # [4, 6400]
    S  = f1^T @ f2                   # [6400, 6400]  (0.5 = 1/sqrt(nclass) folded into f1)
    A  = softmax(S, axis=1)          # row softmax (over q)
    V  = bilinear_resize(out[n])     # [4, 6400]
    fina[c, q] = sum_p V[c, p]/Z_p * exp(S[p, q])

Sharding: 2 batches x 4 p-shards (rows of S) = 8 cores. Each core produces a
partial fina over its 1600 p-rows; host sums the 4 partials per batch.

Device kernel per core (p-shard of 1664 rows incl. pad, all 6400 q).
ScalarE (exp, 1 elem/lane/cycle) is the bottleneck engine; the structure
minimizes ScalarE instruction count and hides everything else under it:
  - S chunk = matmul(lhsT=f1pad[:, pblock(128 cols)], rhs=f2pad[:, qchunk])
    single M=128 PE tile (K=128 keeps fast-weight-load), N=512 chunks into
    PSUM round buffers of 1536 f32 (3 banks, double buffered = 6 banks).
  - exp on ScalarE PSUM->SBUF (fp16 E strip) in 5 rounds/block
    (4x1536 + 256) with accum_out giving row sums Z.
  - fina = matmul(lhsT=vt*recip(Z), rhs=E) col-tiled 4x (q-group g on PE col
    group g) into a 1-bank PSUM scratch (double buffered = 2 banks), then
    DVE-accumulated into an SBUF accumulator, one 512-wide wave per S round
    of the next block so the PE and DVE loads stay smooth.
"""

import numpy as np

N_CORES = 8
NB = 2          # batches
NCLS = 4        # nclass
C_IN = 32
H = W = 80
HW = H * W      # 6400
NSH = 4         # p-shards per batch
PSH = HW // NSH  # 1600 p rows per shard
PBLK = 13        # p blocks of 128 (1664 = 13*128, last 64 rows are zero-pad)
PPAD = PBLK * 128  # 1664
QCH = 512        # q chunk (psum bank)
ROUND = 1536     # q per exp round (3 psum banks)
RWID = (1536, 1536, 1536, 1536, 256)
NROUND = 5

_CACHE = {}


def _resize_bilinear_ac(x, h_out, w_out):
    """numpy mirror of the reference's align_corners=True bilinear resize."""
    n, c, h, w = x.shape
    if (h, w) == (h_out, w_out):
        return x
    ys = np.linspace(0.0, h - 1.0, h_out, dtype=np.float32)
    xs = np.linspace(0.0, w - 1.0, w_out, dtype=np.float32)
    y0 = np.floor(ys).astype(np.int32)
    x0 = np.floor(xs).astype(np.int32)
    y1 = np.minimum(y0 + 1, h - 1)
    x1 = np.minimum(x0 + 1, w - 1)
    wy = (ys - y0.astype(np.float32))[None, None, :, None]
    wx = (xs - x0.astype(np.float32))[None, None, None, :]
    g = lambda yi, xi: x[:, :, yi, :][:, :, :, xi]
    top = g(y0, x0) * (1.0 - wx) + g(y0, x1) * wx
    bot = g(y1, x0) * (1.0 - wx) + g(y1, x1) * wx
    return (top * (1.0 - wy) + bot * wy).astype(np.float32)


def _build_bass():
    import concourse.bass as bass
    import concourse.tile as tile
    from concourse import bacc, mybir

    f32 = mybir.dt.float32
    f16 = mybir.dt.float16
    u32 = mybir.dt.uint32

    nc = bacc.Bacc(
        "TRN2", target_bir_lowering=False, debug=False, num_devices=N_CORES
    )

    f1p_d = nc.dram_tensor("f1p", [12, PPAD], f16, kind="ExternalInput")
    f2p_d = nc.dram_tensor("f2p", [12, HW], f16, kind="ExternalInput")
    vt_d = nc.dram_tensor("vt", [128, NCLS * PBLK], f32, kind="ExternalInput")
    res_d = nc.dram_tensor("res", [4 * NCLS, 4 * QCH], f32, kind="ExternalOutput")

    EXP = mybir.ActivationFunctionType.Exp
    ADD = mybir.AluOpType.add
    MULT = mybir.AluOpType.mult
    AXX = mybir.AxisListType.X

    with tile.TileContext(nc) as tc:
        with (
            tc.tile_pool(name="const", bufs=1) as cpool,
            tc.tile_pool(name="estrip", bufs=2) as epool,
            tc.tile_pool(name="zpool", bufs=2) as zpool,
            tc.tile_pool(name="spsum", bufs=2, space="PSUM") as spool,
            tc.tile_pool(name="fpsum", bufs=1, space="PSUM") as fpool,
            tc.tile_pool(name="s4psum", bufs=1, space="PSUM") as s4pool,
        ):
            # K=128 keeps FWL (fast weight load) active; pad rows 12..127 are
            # zeroed below (u32-bitcast memsets, split across DVE and GpSimd,
            # hidden under the kernel preamble + input DMAs)
            f1s = cpool.tile([128, PPAD], f16, tag="f1s")
            f2s = cpool.tile([128, HW], f16, tag="f2s")
            vts = cpool.tile([128, NCLS * PBLK], f32, tag="vts")
            accsb = cpool.tile([128, 4 * QCH], f32, tag="accsb")
            # ping-pong [128, 32] lhsT tiles for fina (cols 4..31 stay zero)
            vtpA = cpool.tile([128, 32], f16, tag="vtpA")
            vtpB = cpool.tile([128, 32], f16, tag="vtpB")
            bneg = cpool.tile([128, 1], f32, tag="bneg")
            scr = cpool.tile([128, 1], f32, tag="scr")
            # dedicated buffer for the small tail round: keeps the 4 big
            # rounds on a clean A/B rotation, so the next block's first S
            # matmul only waits on exp of round r2 (a full round of slack)
            st4 = s4pool.tile([128, RWID[4]], f32, tag="st4")

            # pad-row zeroing split across GpSimd and DVE so it finishes in
            # ~2us; the ACT table load (dummy exp) overlaps it on ScalarE
            nc.gpsimd.memset(vtpA[:, :], 0.0)
            nc.gpsimd.memset(vtpB[:, :], 0.0)
            nc.gpsimd.memset(bneg[:, :], -5.0)
            nc.gpsimd.memset(f2s[:, HW // 2 :].bitcast(u32), 0)
            nc.vector.memset(f1s[:, :].bitcast(u32), 0)
            nc.vector.memset(f2s[:, 0 : HW // 2].bitcast(u32), 0)
            nc.scalar.activation(scr[:, 0:1], bneg[:, 0:1], EXP)
            # input DMAs: first S round needs f2 cols 0:1536 and f1 only
            nc.sync.dma_start(out=f2s[0:12, 0:ROUND], in_=f2p_d[:, 0:ROUND])
            nc.sync.dma_start(out=f1s[0:12, :], in_=f1p_d[:, :])
            nc.sync.dma_start(out=f2s[0:12, ROUND:], in_=f2p_d[:, ROUND:])
            nc.sync.dma_start(out=vts[:, :], in_=vt_d[:, :])

            ets = [None] * PBLK
            vtps = [None] * PBLK

            def emit_fina_wave(pb, t):
                # col-tiled 4x: group g covers q in [1600g, 1600g+1600) as
                # waves of N=512,512,512,64 into a 1-bank PSUM scratch;
                # group g on partition quadrant 32g. DVE accumulates the
                # scratch into the SBUF accumulator (cols 512t..512t+qw).
                et = ets[pb]
                vtp = vtps[pb]
                qw = 512 if t < 3 else 64
                fsc = fpool.tile([128, QCH], f32, tag="fsc")
                for g in range(4):
                    qo = 1600 * g + QCH * t
                    nc.tensor.matmul(
                        fsc[32 * g : 32 * g + 32, 0:qw],
                        lhsT=vtp[:, :],
                        rhs=et[:, qo : qo + qw],
                        start=True,
                        stop=True,
                        tile_position=(0, 32 * g),
                    )
                dst = accsb[:, QCH * t : QCH * t + qw]
                if pb == 0:
                    nc.vector.tensor_copy(dst, fsc[:, 0:qw])
                else:
                    nc.vector.tensor_add(dst, dst, fsc[:, 0:qw])

            for pb in range(PBLK):
                et = epool.tile([128, HW], f16, tag="et")
                zparts = zpool.tile([128, NROUND], f32, tag="zparts")
                rz = zpool.tile([128, 1], f32, tag="rz")
                vtp = vtpA if pb % 2 == 0 else vtpB
                ets[pb] = et
                vtps[pb] = vtp

                q0 = 0
                for r in range(NROUND):
                    # keep the PE/DVE fed: spread last block's fina waves
                    # across this block's later S rounds
                    if r >= 1 and pb > 0:
                        emit_fina_wave(pb - 1, r - 1)
                    width = RWID[r]
                    st = st4 if r == 4 else spool.tile([128, ROUND], f32, tag="st")
                    for half in range((width + QCH - 1) // QCH):
                        qo = q0 + QCH * half
                        qw = min(QCH, width - QCH * half)
                        nc.tensor.matmul(
                            st[:, QCH * half : QCH * half + qw],
                            lhsT=f1s[:, 128 * pb : 128 * pb + 128],
                            rhs=f2s[:, qo : qo + qw],
                            start=True,
                            stop=True,
                        )
                    # bias -5: keeps exp within fp16 range (softmax is
                    # shift-invariant; Z accumulates the same shifted values)
                    nc.scalar.activation(
                        et[:, q0 : q0 + width],
                        st[:, 0:width],
                        EXP,
                        bias=bneg[:, 0:1],
                        accum_out=zparts[:, r : r + 1],
                    )
                    q0 += width

                # Z = sum of round partials; vtp = vt[:, block] / Z
                nc.vector.tensor_reduce(rz[:, 0:1], zparts[:, :], AXX, ADD)
                nc.vector.reciprocal(rz[:, 0:1], rz[:, 0:1])
                nc.vector.tensor_scalar(
                    vtp[:, 0:NCLS],
                    vts[:, NCLS * pb : NCLS * pb + NCLS],
                    rz[:, 0:1],
                    2048.0,
                    MULT,
                    MULT,
                )

            # result lands in SBUF as the last block's waves complete; cols
            # 0:1536 are final after wave 2's add, so their DMAs are emitted
            # before wave 3 to overlap it. Issues spread over the
            # sync/scalar/gpsimd queues so descriptor setup overlaps.
            dma_engines = [nc.sync, nc.scalar, nc.gpsimd, nc.sync]
            for t in range(3):
                emit_fina_wave(PBLK - 1, t)
            for g in range(4):
                dma_engines[g].dma_start(
                    out=res_d[NCLS * g : NCLS * g + NCLS, 0 : 3 * QCH],
                    in_=accsb[32 * g : 32 * g + NCLS, 0 : 3 * QCH],
                )
            emit_fina_wave(PBLK - 1, 3)
            for g in range(4):
                dma_engines[(g + 1) % 3].dma_start(
                    out=res_d[NCLS * g : NCLS * g + NCLS, 3 * QCH : PSH],
                    in_=accsb[32 * g : 32 * g + NCLS, 3 * QCH : PSH],
                )

    nc.compile()
    return nc


def _get_nc():
    if "nc" not in _CACHE:
        _CACHE["nc"] = _build_bass()
    return _CACHE["nc"]


def _hilo16(x):
    """fp16 high/low split: x ~= hi + lo exactly to ~2^-22 relative."""
    x = np.asarray(x, np.float32)
    hi = x.astype(np.float16)
    lo = (x - hi.astype(np.float32)).astype(np.float16)
    return hi, lo


def _prep_inputs(feature_in, out, w1, b1, w2, b2):
    feature_in = np.asarray(feature_in, np.float32)
    out = np.asarray(out, np.float32)
    w1 = np.asarray(w1, np.float32)
    b1 = np.asarray(b1, np.float32)
    w2 = np.asarray(w2, np.float32)
    b2 = np.asarray(b2, np.float32)

    scale = np.float32(1.0 / np.sqrt(NCLS))
    feat = feature_in.reshape(NB, C_IN, HW)
    # f1 carries the softmax scale; f2 is plain
    f1 = (np.einsum("oc,ncp->nop", w1, feat, dtype=np.float32) + b1[None, :, None]) * scale
    f2 = np.einsum("oc,ncp->nop", w2, feat, dtype=np.float32) + b2[None, :, None]
    f1 = f1.astype(np.float32)
    f2 = f2.astype(np.float32)
    v = _resize_bilinear_ac(out, H, W).reshape(NB, NCLS, HW)

    in_maps = []
    for core in range(N_CORES):
        b, s = divmod(core, NSH)
        p0 = PSH * s
        f1p = np.zeros((12, PPAD), np.float16)
        h1, l1 = _hilo16(f1[b][:, p0 : p0 + PSH])
        f1p[0:4, :PSH] = h1
        f1p[4:8, :PSH] = l1
        f1p[8:12, :PSH] = h1
        h2, l2 = _hilo16(f2[b])
        f2p = np.concatenate([h2, h2, l2], axis=0)  # [12, HW] fp16
        vtp = np.zeros((NCLS, PPAD), np.float32)
        vtp[:, :PSH] = v[b][:, p0 : p0 + PSH]
        # vt[part, 4*pb + c] = V[c, p0 + 128*pb + part]
        vt = vtp.reshape(NCLS, PBLK, 128).transpose(2, 1, 0).reshape(128, PBLK * NCLS)
        in_maps.append(
            {
                "f1p": f1p,
                "f2p": np.ascontiguousarray(f2p),
                "vt": np.ascontiguousarray(vt),
            }
        )
    return in_maps


def _unpack(results):
    """results: list of 8 dicts with 'res' [16, 2048] -> fina [2,4,80,80]."""
    fina = np.zeros((NB, NCLS, HW), np.float32)
    for core in range(N_CORES):
        b, s = divmod(core, NSH)
        res = np.asarray(results[core]["res"], np.float32)  # [16, 2048]
        part = res.reshape(4, NCLS, 4 * QCH)  # [q-group g, class j, cols]
        for g in range(4):
            fina[b, :, PSH * g : PSH * g + PSH] += part[g][:, :PSH]
    fina *= np.float32(1.0 / 2048.0)
    return fina.reshape(NB, NCLS, H, W)


def run(inputs, trace=False):
    from concourse.bass_utils import run_bass_kernel_spmd

    nc = _get_nc()
    in_maps = _prep_inputs(**inputs)
    r = run_bass_kernel_spmd(nc, in_maps, list(range(N_CORES)), trace=trace)
    return _unpack(r.results), r.exec_time_ns


def kernel(feature_in, out, w1, b1, w2, b2):
    result, _ = run(
        dict(feature_in=feature_in, out=out, w1=w1, b1=b1, w2=w2, b2=b2)
    )
    return result


# revision 24
# speedup vs baseline: 1.2306x; 1.0160x over previous
"""Trainium2 Bass kernel for nn_Corr (correlation-attention module).

Math (per batch n):
    f1 = 0.5*(w1 @ feat + b1)        # [4, 6400]   feat = feature_in[n] flattened
    f2 =      w2 @ feat + b2         # [4, 6400]
    S  = f1^T @ f2                   # [6400, 6400]  (0.5 = 1/sqrt(nclass) folded into f1)
    A  = softmax(S, axis=1)          # row softmax (over q)
    V  = bilinear_resize(out[n])     # [4, 6400]
    fina[c, q] = sum_p V[c, p]/Z_p * exp(S[p, q])

Sharding: 2 batches x 4 p-shards (rows of S) = 8 cores. Each core produces a
partial fina over its 1600 p-rows; host sums the 4 partials per batch.

Device kernel per core (p-shard of 1664 rows incl. pad, all 6400 q).
ScalarE (exp, 1 elem/lane/cycle) is the bottleneck engine; the structure
minimizes ScalarE instruction count and hides everything else under it:
  - S chunk = matmul(lhsT=f1pad[:, pblock(128 cols)], rhs=f2pad[:, qchunk])
    single M=128 PE tile (K=128 keeps fast-weight-load), N=512 chunks into
    PSUM round buffers of 1536 f32 (3 banks, double buffered = 6 banks).
  - exp on ScalarE PSUM->SBUF (fp16 E strip) in 5 rounds/block
    (4x1536 + 256) with accum_out giving row sums Z.
  - fina = matmul(lhsT=vt*recip(Z), rhs=E) col-tiled 4x (q-group g on PE col
    group g) into a 1-bank PSUM scratch (double buffered = 2 banks), then
    DVE-accumulated into an SBUF accumulator, one 512-wide wave per S round
    of the next block so the PE and DVE loads stay smooth.
"""

import numpy as np

N_CORES = 8
NB = 2          # batches
NCLS = 4        # nclass
C_IN = 32
H = W = 80
HW = H * W      # 6400
NSH = 4         # p-shards per batch
PSH = HW // NSH  # 1600 p rows per shard
PBLK = 13        # p blocks of 128 (1664 = 13*128, last 64 rows are zero-pad)
PPAD = PBLK * 128  # 1664
QCH = 512        # q chunk (psum bank)
ROUND = 1536     # q per exp round (3 psum banks)
RWID = (1536, 1536, 1536, 1536, 256)
NROUND = 5

_CACHE = {}


def _resize_bilinear_ac(x, h_out, w_out):
    """numpy mirror of the reference's align_corners=True bilinear resize."""
    n, c, h, w = x.shape
    if (h, w) == (h_out, w_out):
        return x
    ys = np.linspace(0.0, h - 1.0, h_out, dtype=np.float32)
    xs = np.linspace(0.0, w - 1.0, w_out, dtype=np.float32)
    y0 = np.floor(ys).astype(np.int32)
    x0 = np.floor(xs).astype(np.int32)
    y1 = np.minimum(y0 + 1, h - 1)
    x1 = np.minimum(x0 + 1, w - 1)
    wy = (ys - y0.astype(np.float32))[None, None, :, None]
    wx = (xs - x0.astype(np.float32))[None, None, None, :]
    g = lambda yi, xi: x[:, :, yi, :][:, :, :, xi]
    top = g(y0, x0) * (1.0 - wx) + g(y0, x1) * wx
    bot = g(y1, x0) * (1.0 - wx) + g(y1, x1) * wx
    return (top * (1.0 - wy) + bot * wy).astype(np.float32)


def _build_bass():
    import concourse.bass as bass
    import concourse.tile as tile
    from concourse import bacc, mybir

    f32 = mybir.dt.float32
    f16 = mybir.dt.float16
    u32 = mybir.dt.uint32

    nc = bacc.Bacc(
        "TRN2", target_bir_lowering=False, debug=False, num_devices=N_CORES
    )

    f1p_d = nc.dram_tensor("f1p", [12, PPAD], f16, kind="ExternalInput")
    f2p_d = nc.dram_tensor("f2p", [12, HW], f16, kind="ExternalInput")
    vt_d = nc.dram_tensor("vt", [128, NCLS * PBLK], f32, kind="ExternalInput")
    res_d = nc.dram_tensor("res", [4 * NCLS, 4 * QCH], f32, kind="ExternalOutput")

    EXP = mybir.ActivationFunctionType.Exp
    ADD = mybir.AluOpType.add
    MULT = mybir.AluOpType.mult
    AXX = mybir.AxisListType.X

    with tile.TileContext(nc) as tc:
        with (
            tc.tile_pool(name="const", bufs=1) as cpool,
            tc.tile_pool(name="estrip", bufs=2) as epool,
            tc.tile_pool(name="zpool", bufs=2) as zpool,
            tc.tile_pool(name="spsum", bufs=2, space="PSUM") as spool,
            tc.tile_pool(name="fpsum", bufs=1, space="PSUM") as fpool,
            tc.tile_pool(name="s4psum", bufs=1, space="PSUM") as s4pool,
        ):
            # K=128 keeps FWL (fast weight load) active; pad rows 12..127 are
            # zeroed below (u32-bitcast memsets, split across DVE and GpSimd,
            # hidden under the kernel preamble + input DMAs)
            f1s = cpool.tile([128, PPAD], f16, tag="f1s")
            f2s = cpool.tile([128, HW], f16, tag="f2s")
            vts = cpool.tile([128, NCLS * PBLK], f32, tag="vts")
            accsb = cpool.tile([128, 4 * QCH], f32, tag="accsb")
            # ping-pong [128, 32] lhsT tiles for fina (cols 4..31 stay zero)
            vtpA = cpool.tile([128, 32], f16, tag="vtpA")
            vtpB = cpool.tile([128, 32], f16, tag="vtpB")
            bneg = cpool.tile([128, 1], f32, tag="bneg")
            scr = cpool.tile([128, 1], f32, tag="scr")
            # dedicated buffer for the small tail round: keeps the 4 big
            # rounds on a clean A/B rotation, so the next block's first S
            # matmul only waits on exp of round r2 (a full round of slack)
            st4 = s4pool.tile([128, RWID[4]], f32, tag="st4")

            # pad-row zeroing split across GpSimd/DVE/ScalarE so it finishes
            # ~1.5us after the preamble; the ACT table load (dummy exp)
            # overlaps it, and the first S round's inputs DMA in first
            nc.gpsimd.memset(vtpA[:, :], 0.0)
            nc.gpsimd.memset(vtpB[:, :], 0.0)
            nc.gpsimd.memset(bneg[:, :], -5.0)
            nc.vector.memset(f1s[:, :].bitcast(u32), 0)
            nc.gpsimd.memset(f2s[:, 0 : ROUND * 2].bitcast(u32), 0)
            nc.vector.memset(f2s[:, ROUND * 2 :].bitcast(u32), 0)
            nc.scalar.activation(scr[:, 0:1], bneg[:, 0:1], EXP)
            nc.sync.dma_start(out=f1s[0:12, :], in_=f1p_d[:, :])
            nc.sync.dma_start(out=f2s[0:12, 0:ROUND], in_=f2p_d[:, 0:ROUND])
            nc.sync.dma_start(out=f2s[0:12, ROUND:], in_=f2p_d[:, ROUND:])
            nc.sync.dma_start(out=vts[:, :], in_=vt_d[:, :])

            ets = [None] * PBLK
            vtps = [None] * PBLK

            def emit_fina_wave(pb, t, fsc=None):
                # col-tiled 4x: group g covers q in [1600g, 1600g+1600) as
                # waves of N=512,512,512,64 into a 1-bank PSUM scratch;
                # group g on partition quadrant 32g. DVE accumulates the
                # scratch into the SBUF accumulator (cols 512t..512t+qw).
                et = ets[pb]
                vtp = vtps[pb]
                qw = 512 if t < 3 else 64
                if fsc is None:
                    fsc = fpool.tile([128, QCH], f32, tag="fsc")
                for g in range(4):
                    qo = 1600 * g + QCH * t
                    nc.tensor.matmul(
                        fsc[32 * g : 32 * g + 32, 0:qw],
                        lhsT=vtp[:, :],
                        rhs=et[:, qo : qo + qw],
                        start=True,
                        stop=True,
                        tile_position=(0, 32 * g),
                    )
                dst = accsb[:, QCH * t : QCH * t + qw]
                if pb == 0:
                    nc.vector.tensor_copy(dst, fsc[:, 0:qw])
                else:
                    nc.vector.tensor_add(dst, dst, fsc[:, 0:qw])

            for pb in range(PBLK):
                et = epool.tile([128, HW], f16, tag="et")
                zparts = zpool.tile([128, NROUND], f32, tag="zparts")
                rz = zpool.tile([128, 1], f32, tag="rz")
                vtp = vtpA if pb % 2 == 0 else vtpB
                ets[pb] = et
                vtps[pb] = vtp

                q0 = 0
                for r in range(NROUND):
                    # keep the PE/DVE fed: spread last block's fina waves
                    # across this block's later S rounds
                    if r >= 1 and pb > 0:
                        emit_fina_wave(pb - 1, r - 1)
                    width = RWID[r]
                    st = st4 if r == 4 else spool.tile([128, ROUND], f32, tag="st")
                    for half in range((width + QCH - 1) // QCH):
                        qo = q0 + QCH * half
                        qw = min(QCH, width - QCH * half)
                        nc.tensor.matmul(
                            st[:, QCH * half : QCH * half + qw],
                            lhsT=f1s[:, 128 * pb : 128 * pb + 128],
                            rhs=f2s[:, qo : qo + qw],
                            start=True,
                            stop=True,
                        )
                    # bias -5: keeps exp within fp16 range (softmax is
                    # shift-invariant; Z accumulates the same shifted values)
                    nc.scalar.activation(
                        et[:, q0 : q0 + width],
                        st[:, 0:width],
                        EXP,
                        bias=bneg[:, 0:1],
                        accum_out=zparts[:, r : r + 1],
                    )
                    q0 += width

                # Z = sum of round partials; vtp = vt[:, block] / Z
                nc.vector.tensor_reduce(rz[:, 0:1], zparts[:, :], AXX, ADD)
                nc.vector.reciprocal(rz[:, 0:1], rz[:, 0:1])
                nc.vector.tensor_scalar(
                    vtp[:, 0:NCLS],
                    vts[:, NCLS * pb : NCLS * pb + NCLS],
                    rz[:, 0:1],
                    2048.0,
                    MULT,
                    MULT,
                )

            # result lands in SBUF as the last block's waves complete; cols
            # 0:1536 are final after wave 2's add, so their DMAs are emitted
            # before wave 3 to overlap it. Issues spread over the
            # sync/scalar/gpsimd queues so descriptor setup overlaps.
            # the tail waves borrow the (now idle) spool double buffer so
            # wave t+1's matmuls overlap wave t's DVE add
            dma_engines = [nc.sync, nc.scalar, nc.gpsimd, nc.sync]
            for t in range(3):
                tw = spool.tile([128, ROUND], f32, tag="st")
                emit_fina_wave(PBLK - 1, t, fsc=tw)
            for g in range(4):
                dma_engines[g].dma_start(
                    out=res_d[NCLS * g : NCLS * g + NCLS, 0 : 3 * QCH],
                    in_=accsb[32 * g : 32 * g + NCLS, 0 : 3 * QCH],
                )
            tw = spool.tile([128, ROUND], f32, tag="st")
            emit_fina_wave(PBLK - 1, 3, fsc=tw)
            for g in range(4):
                dma_engines[(g + 1) % 3].dma_start(
                    out=res_d[NCLS * g : NCLS * g + NCLS, 3 * QCH : PSH],
                    in_=accsb[32 * g : 32 * g + NCLS, 3 * QCH : PSH],
                )

    nc.compile()
    return nc


def _get_nc():
    if "nc" not in _CACHE:
        _CACHE["nc"] = _build_bass()
    return _CACHE["nc"]


def _hilo16(x):
    """fp16 high/low split: x ~= hi + lo exactly to ~2^-22 relative."""
    x = np.asarray(x, np.float32)
    hi = x.astype(np.float16)
    lo = (x - hi.astype(np.float32)).astype(np.float16)
    return hi, lo


def _prep_inputs(feature_in, out, w1, b1, w2, b2):
    feature_in = np.asarray(feature_in, np.float32)
    out = np.asarray(out, np.float32)
    w1 = np.asarray(w1, np.float32)
    b1 = np.asarray(b1, np.float32)
    w2 = np.asarray(w2, np.float32)
    b2 = np.asarray(b2, np.float32)

    scale = np.float32(1.0 / np.sqrt(NCLS))
    feat = feature_in.reshape(NB, C_IN, HW)
    # f1 carries the softmax scale; f2 is plain
    f1 = (np.einsum("oc,ncp->nop", w1, feat, dtype=np.float32) + b1[None, :, None]) * scale
    f2 = np.einsum("oc,ncp->nop", w2, feat, dtype=np.float32) + b2[None, :, None]
    f1 = f1.astype(np.float32)
    f2 = f2.astype(np.float32)
    v = _resize_bilinear_ac(out, H, W).reshape(NB, NCLS, HW)

    in_maps = []
    for core in range(N_CORES):
        b, s = divmod(core, NSH)
        p0 = PSH * s
        f1p = np.zeros((12, PPAD), np.float16)
        h1, l1 = _hilo16(f1[b][:, p0 : p0 + PSH])
        f1p[0:4, :PSH] = h1
        f1p[4:8, :PSH] = l1
        f1p[8:12, :PSH] = h1
        h2, l2 = _hilo16(f2[b])
        f2p = np.concatenate([h2, h2, l2], axis=0)  # [12, HW] fp16
        vtp = np.zeros((NCLS, PPAD), np.float32)
        vtp[:, :PSH] = v[b][:, p0 : p0 + PSH]
        # vt[part, 4*pb + c] = V[c, p0 + 128*pb + part]
        vt = vtp.reshape(NCLS, PBLK, 128).transpose(2, 1, 0).reshape(128, PBLK * NCLS)
        in_maps.append(
            {
                "f1p": f1p,
                "f2p": np.ascontiguousarray(f2p),
                "vt": np.ascontiguousarray(vt),
            }
        )
    return in_maps


def _unpack(results):
    """results: list of 8 dicts with 'res' [16, 2048] -> fina [2,4,80,80]."""
    fina = np.zeros((NB, NCLS, HW), np.float32)
    for core in range(N_CORES):
        b, s = divmod(core, NSH)
        res = np.asarray(results[core]["res"], np.float32)  # [16, 2048]
        part = res.reshape(4, NCLS, 4 * QCH)  # [q-group g, class j, cols]
        for g in range(4):
            fina[b, :, PSH * g : PSH * g + PSH] += part[g][:, :PSH]
    fina *= np.float32(1.0 / 2048.0)
    return fina.reshape(NB, NCLS, H, W)


def run(inputs, trace=False):
    from concourse.bass_utils import run_bass_kernel_spmd

    nc = _get_nc()
    in_maps = _prep_inputs(**inputs)
    r = run_bass_kernel_spmd(nc, in_maps, list(range(N_CORES)), trace=trace)
    return _unpack(r.results), r.exec_time_ns


def kernel(feature_in, out, w1, b1, w2, b2):
    result, _ = run(
        dict(feature_in=feature_in, out=out, w1=w1, b1=b1, w2=w2, b2=b2)
    )
    return result
